# revision 1
# baseline (speedup 1.0000x reference)
"""GraphSAGE 2-block GNN (nn_BaselineModel_80607946211554) on 8 TRN2 NeuronCores.

Strategy: destination-node sharding. Each core owns a contiguous range of
6250 nodes. Node-feature tables (x, and intermediate h tables) are replicated
in each core's DRAM in a "slab" layout: node n -> table row (n//6250)*6272 +
n%6250, with 22 zero pad rows per slab. Neighbor aggregation is done by
dma_gather of source rows (edges sorted by destination, host-preprocessed)
followed by a fixed ones-block matmul (sums blocks of 4 slots, col-tiled on
the PE array) and a per-window indicator matmul mapping blocks to
destinations. SAGE linear layers run feature-major (weights stationary on the
PE). Intermediate node tables are rebuilt across cores with AllGather
collectives. Graph pooling is a one-hot matmul; the tiny MLP head + softmax is
computed redundantly on every core.

Self-contained: hardcodes all shapes for the fixed problem instance.
"""
import os
import sys
import types
import numpy as np

N = 50000
E = 1600000
G = 256
F = 128
HID = 128
C = 10
NCORES = 8
NPC = N // NCORES            # 6250 nodes per core
SLAB = 6272                  # slab rows (6250 + 22 zero pad)
NT = NCORES * SLAB           # 50176 table rows
LO = 4 * SLAB                # 25088; table rows < LO hold nodes < 25000
PADROW = 6250                # zero row (local index in both lo/hi views)
P = 128
NW = (NPC + P - 1) // P      # 49 dst windows per core
EPS = 1e-5

_prog_cache = {}


# ----------------------------------------------------------------- host prep
def _wrap_idx(sec):
    """int64 slot values (len mult of 16) -> [128, n/16] int16 wrapped layout."""
    n = len(sec)
    arr = sec.reshape(n // 16, 16).T.astype(np.int16)   # [16, n/16]
    return np.tile(arr, (8, 1))                          # [128, n/16]


def _build_schedule(src, dst, invd_full):
    """Shared static schedule + per-core gather/indicator data.
    ind2 entries carry 1/deg(dst) so stage-2 emits the neighbourhood mean."""
    core_edges = []
    SL = np.zeros((NCORES, NW), np.int64)
    SH = np.zeros((NCORES, NW), np.int64)
    for c in range(NCORES):
        m = (dst >= c * NPC) & (dst < (c + 1) * NPC)
        s = src[m].astype(np.int64)
        d = (dst[m] - c * NPC).astype(np.int64)
        hi = (s >= N // 2).astype(np.int64)
        w = d >> 7
        order = np.lexsort((d, hi, w))
        s, d, hi, w = s[order], d[order], hi[order], w[order]
        core_edges.append((s, d, hi, w))
        cnt = np.bincount(d * 2 + hi, minlength=NPC * 2).reshape(NPC, 2)
        pl = ((cnt + 3) >> 2) << 2
        plp = np.zeros((NW * P, 2), np.int64)
        plp[:NPC] = pl
        plw = plp.reshape(NW, P, 2).sum(1)
        SL[c], SH[c] = plw[:, 0], plw[:, 1]

    nL = np.maximum(((SL.max(0) + 127) // P) * P, P).astype(np.int64)
    nH = np.maximum(((SH.max(0) + 127) // P) * P, P).astype(np.int64)
    S = nL + nH
    B = S // 4
    T = (B + 127) // P
    colL = np.zeros(NW, np.int64)
    colH = np.zeros(NW, np.int64)
    off = 0
    for w in range(NW):
        colL[w] = off
        off += nL[w] // 16
        colH[w] = off
        off += nH[w] // 16
    idx_cols = off
    i2off = np.zeros(NW, np.int64)
    o = 0
    for w in range(NW):
        i2off[w] = o
        o += T[w] * P
    i2rows = o

    sched = dict(nL=nL, nH=nH, S=S, B=B, T=T, colL=colL, colH=colH,
                 idx_cols=idx_cols, i2off=i2off, i2rows=i2rows,
                 smax=int(S.max()))

    per_core = []
    for c in range(NCORES):
        s, d, hi, w = core_edges[c]
        cnt = np.bincount(d * 2 + hi, minlength=NPC * 2).reshape(NPC, 2)
        pl = ((cnt + 3) >> 2) << 2
        plp = np.zeros((NW * P, 2), np.int64)
        plp[:NPC] = pl
        plw3 = plp.reshape(NW, P, 2)
        gstart = np.cumsum(plw3, axis=1) - plw3           # [NW,128,2]
        key = d * 2 + hi
        if len(key):
            grp_change = np.r_[True, key[1:] != key[:-1]]
            gidx = np.cumsum(grp_change) - 1
            first_pos = np.flatnonzero(grp_change)
            rank = np.arange(len(d)) - first_pos[gidx]
        else:
            rank = np.zeros(0, np.int64)
        pos = gstart[w, d & 127, hi] + rank
        trow = (s // NPC) * SLAB + s % NPC
        val = np.where(hi == 1, trow - LO, trow)

        idx_arr = np.zeros((P, idx_cols), np.int16)
        ind2 = np.zeros((i2rows, P), np.float32)
        for wi in range(NW):
            mw = w == wi
            mL = mw & (hi == 0)
            mH = mw & (hi == 1)
            secL = np.full(nL[wi], PADROW, np.int64)
            secH = np.full(nH[wi], PADROW, np.int64)
            secL[pos[mL]] = val[mL]
            secH[pos[mH]] = val[mH]
            idx_arr[:, colL[wi]:colL[wi] + nL[wi] // 16] = _wrap_idx(secL)
            idx_arr[:, colH[wi]:colH[wi] + nH[wi] // 16] = _wrap_idx(secH)
            d0, d1 = wi * P, min((wi + 1) * P, NPC)
            dloc = np.arange(d1 - d0)
            bL = np.repeat(dloc, pl[d0:d1, 0] // 4)
            bH = np.repeat(dloc, pl[d0:d1, 1] // 4)
            b2d = np.full(T[wi] * P, -1, np.int64)
            b2d[:len(bL)] = bL
            b2d[nL[wi] // 4:nL[wi] // 4 + len(bH)] = bH
            rows = np.arange(T[wi] * P)
            vmask = b2d >= 0
            blk = ind2[i2off[wi]:i2off[wi] + T[wi] * P]
            blk[rows[vmask], b2d[vmask]] = invd_full[c * NPC + wi * P + b2d[vmask]]
        per_core.append(dict(idx=idx_arr, ind2=ind2))
    return sched, per_core


def _host_inputs(inputs):
    f32 = lambda a: np.asarray(a, np.float32)
    x = f32(inputs["x"])
    ei = np.asarray(inputs["edge_index"], np.int64)
    batch = np.asarray(inputs["batch"], np.int64)
    src, dst = ei[0], ei[1]

    deg = np.bincount(dst, minlength=N).astype(np.float32)
    invd_full = (1.0 / np.maximum(deg, 1.0)).astype(np.float32)

    sched, per_core = _build_schedule(src, dst, invd_full)

    xt = np.zeros((NT, F), np.float32)
    for r in range(NCORES):
        xt[r * SLAB:r * SLAB + NPC] = x[r * NPC:(r + 1) * NPC]

    o4 = np.zeros((P, 32), np.float32)
    for e in range(P):
        o4[e, e // 4] = 1.0
    ident = np.eye(P, dtype=np.float32)

    # BN folding
    s_bn = f32(inputs["bn_gamma"]) / np.sqrt(f32(inputs["bn_rv"]) + EPS)
    t_bn = f32(inputs["bn_beta"]) - f32(inputs["bn_rm"]) * s_bn
    bns2 = s_bn.reshape(2, P).T.copy()     # [128, 2]
    bnt2 = t_bn.reshape(2, P).T.copy()

    shared = {
        "xt": xt, "o4": o4, "ident": ident,
        "bns2": bns2, "bnt2": bnt2,
        "l1w": f32(inputs["lin1_W"]), "l1b": f32(inputs["lin1_b"]),
        "l2w": f32(inputs["lin2_W"]), "l2b": f32(inputs["lin2_b"]),
    }
    for b in (0, 1):
        for nm in ("Wl1", "Wr1", "b1", "Wl2", "Wr2", "b2", "Wlin", "blin"):
            shared[f"b{b}_{nm}"] = f32(inputs[f"b{b}_{nm}"])

    in_maps = []
    for c in range(NCORES):
        xoT = np.zeros((F, SLAB), np.float32)
        xoT[:, :NPC] = x[c * NPC:(c + 1) * NPC].T
        invd = np.ones((P, NW), np.float32)
        iv = invd_full[c * NPC:(c + 1) * NPC]
        ivp = np.ones(NW * P, np.float32)
        ivp[:NPC] = iv
        invd[:] = ivp.reshape(NW, P).T
        pool_ind = np.zeros((NW, P, G), np.float32)
        bt = batch[c * NPC:(c + 1) * NPC]
        btp = np.full(NW * P, -1, np.int64)
        btp[:NPC] = bt
        btp2 = btp.reshape(NW, P)
        for wi in range(NW):
            vm = btp2[wi] >= 0
            pool_ind[wi, np.arange(P)[vm], btp2[wi][vm]] = 1.0
        im = dict(shared)
        im.update({
            "xoT": xoT, "invd": invd, "poolind": pool_ind,
            "idx": per_core[c]["idx"], "ind2": per_core[c]["ind2"],
        })
        in_maps.append(im)
    return sched, in_maps


# ------------------------------------------------------------- bass program
def _build_program(sched, n_convs=4, debug_tables=False):
    import concourse.bass as bass
    import concourse.mybir as mybir
    import concourse.tile as tile
    from concourse import bacc
    from concourse import library_config
    from contextlib import ExitStack

    dt = mybir.dt
    DT = dt.float32
    Alu = mybir.AluOpType

    nL, nH, S, B, T = (sched[k] for k in ("nL", "nH", "S", "B", "T"))
    colL, colH, i2off = sched["colL"], sched["colH"], sched["i2off"]
    SMAX = sched["smax"]

    nc = bacc.Bacc("TRN2", debug=False, num_swdge_queues=4)

    # ---- parameters
    xt = nc.declare_dram_parameter("xt", [NT, F], DT, isOutput=False)
    xoT = nc.declare_dram_parameter("xoT", [F, SLAB], DT, isOutput=False)
    idxp = nc.declare_dram_parameter("idx", [P, sched["idx_cols"]], dt.int16, isOutput=False)
    ind2p = nc.declare_dram_parameter("ind2", [sched["i2rows"], P], DT, isOutput=False)
    invdp = nc.declare_dram_parameter("invd", [P, NW], DT, isOutput=False)
    poolp = nc.declare_dram_parameter("poolind", [NW, P, G], DT, isOutput=False)
    o4p = nc.declare_dram_parameter("o4", [P, 32], DT, isOutput=False)
    identp = nc.declare_dram_parameter("ident", [P, P], DT, isOutput=False)
    wp = {}
    for b in (0, 1):
        for nm, shp in (("Wl1", [F, HID]), ("Wr1", [F, HID]), ("b1", [HID]),
                        ("Wl2", [HID, HID]), ("Wr2", [HID, HID]), ("b2", [HID]),
                        ("Wlin", [2 * HID, HID]), ("blin", [HID])):
            wp[f"b{b}_{nm}"] = nc.declare_dram_parameter(f"b{b}_{nm}", shp, DT, isOutput=False)
    bns2p = nc.declare_dram_parameter("bns2", [P, 2], DT, isOutput=False)
    bnt2p = nc.declare_dram_parameter("bnt2", [P, 2], DT, isOutput=False)
    l1wp = nc.declare_dram_parameter("l1w", [2 * HID, HID], DT, isOutput=False)
    l1bp = nc.declare_dram_parameter("l1b", [HID], DT, isOutput=False)
    l2wp = nc.declare_dram_parameter("l2w", [HID, C], DT, isOutput=False)
    l2bp = nc.declare_dram_parameter("l2b", [C], DT, isOutput=False)

    out = nc.declare_dram_parameter("out", [G, C], DT, isOutput=True)
    if debug_tables:
        dbgA = nc.declare_dram_parameter("dbgA", [NT, F], DT, isOutput=True)
        dbgB = nc.declare_dram_parameter("dbgB", [NT, F], DT, isOutput=True)

    with tile.TileContext(nc) as tc, ExitStack() as ctx:
        sb = ctx.enter_context(tc.tile_pool(name="sb", bufs=1))
        sb_feat = ctx.enter_context(tc.tile_pool(name="sb_feat", bufs=1))
        sb_g = ctx.enter_context(tc.tile_pool(name="sb_g", bufs=3))
        sb_idx = ctx.enter_context(tc.tile_pool(name="sb_idx", bufs=6))
        sb_i2 = ctx.enter_context(tc.tile_pool(name="sb_i2", bufs=4))
        sb_bs = ctx.enter_context(tc.tile_pool(name="sb_bs", bufs=4))
        sb_ms = ctx.enter_context(tc.tile_pool(name="sb_ms", bufs=3))
        ps_bs = ctx.enter_context(tc.tile_pool(name="ps_bs", bufs=3, space="PSUM"))
        ps_agg = ctx.enter_context(tc.tile_pool(name="ps_agg", bufs=2, space="PSUM"))
        ps_mm = ctx.enter_context(tc.tile_pool(name="ps_mm", bufs=2, space="PSUM"))
        ps_pool = ctx.enter_context(tc.tile_pool(name="ps_pool", bufs=1, space="PSUM"))
        dram = ctx.enter_context(tc.tile_pool(name="dram", bufs=1, space="DRAM"))

        nc.gpsimd.load_library(library_config.mlp)

        # ---- constants into SBUF
        o4_t = sb.tile([P, 32], DT)
        nc.sync.dma_start(o4_t[:], o4p[:])
        id_t = sb.tile([P, P], DT)
        nc.sync.dma_start(id_t[:], identp[:])
        invd_t = sb.tile([P, NW], DT)
        nc.sync.dma_start(invd_t[:], invdp[:])
        wt = {}
        for b in (0, 1):
            for nm in ("Wl1", "Wr1", "Wl2", "Wr2"):
                w_t = sb.tile([P, P], DT, name=f"w{b}{nm}")
                nc.sync.dma_start(w_t[:], wp[f"b{b}_{nm}"][:])
                wt[f"b{b}_{nm}"] = w_t
            wlin_t = sb.tile([P, 2, P], DT, name=f"w{b}lin")
            nc.sync.dma_start(wlin_t[:, 0, :], wp[f"b{b}_Wlin"][0:P, :])
            nc.sync.dma_start(wlin_t[:, 1, :], wp[f"b{b}_Wlin"][P:2 * P, :])
            wt[f"b{b}_Wlin"] = wlin_t
            for nm in ("b1", "b2", "blin"):
                b_t = sb.tile([P, 1], DT, name=f"b{b}{nm}")
                nc.sync.dma_start(b_t[:], wp[f"b{b}_{nm}"][:, None])
                wt[f"b{b}_{nm}"] = b_t
        bns_t = sb.tile([P, 2], DT)
        nc.sync.dma_start(bns_t[:], bns2p[:])
        bnt_t = sb.tile([P, 2], DT)
        nc.sync.dma_start(bnt_t[:], bnt2p[:])
        l1w_t = sb.tile([P, 2, P], DT)
        nc.sync.dma_start(l1w_t[:, 0, :], l1wp[0:P, :])
        nc.sync.dma_start(l1w_t[:, 1, :], l1wp[P:2 * P, :])
        l1b_t = sb.tile([P, 1], DT)
        nc.sync.dma_start(l1b_t[:], l1bp[:, None])
        l2w_t = sb.tile([P, C], DT)
        nc.sync.dma_start(l2w_t[:], l2wp[:])
        l2b_t = sb.tile([P, 1], DT)
        nc.sync.dma_start(l2b_t[0:C, :], l2bp[:, None])

        # feature-major activation buffers [128, SLAB]
        featA = sb_feat.tile([P, SLAB], DT)   # x_ownT, later h (block0 out), h' ...
        featB = sb_feat.tile([P, SLAB], DT)   # h1, h1'
        featC = sb_feat.tile([P, SLAB], DT)   # h2, h2'
        nc.sync.dma_start(featA[:], xoT[:])

        zero_t = sb.tile([P, P], DT)
        nc.vector.memset(zero_t[:], 0.0)

        # DRAM scratch
        cA = dram.tile([SLAB, F], DT)
        cB = dram.tile([SLAB, F], DT)
        tabA = dram.tile([NT, F], DT, addr_space="Shared")
        tabB = dram.tile([NT, F], DT, addr_space="Shared")
        tabC = dram.tile([NT, F], DT, addr_space="Shared")
        pc_in = dram.tile([P, 2 * G], DT)
        pc_out = dram.tile([P, 2 * G], DT, addr_space="Shared")
        nc.sync.dma_start(cA[NPC:SLAB, :], zero_t[0:SLAB - NPC, :])
        nc.sync.dma_start(cB[NPC:SLAB, :], zero_t[0:SLAB - NPC, :])

        def conv(tab, in_feat, out_feat, Wl, Wr, bcol, contrib):
            """One SAGE conv: out_feat[:, n] = relu(mean@Wl + in@Wr + b).
            If contrib is not None also write node-major rows there."""
            if not hasattr(conv, "qctr"):
                conv.qctr = 0
            for w in range(NW):
                nLw, nHw, Sw, Bw, Tw = int(nL[w]), int(nH[w]), int(S[w]), int(B[w]), int(T[w])
                g_t = sb_g.tile([P, SMAX // P, P], DT, name="g_t")
                CH = 1536
                for sec, nsec, col0, slot0 in (("L", nLw, int(colL[w]), 0),
                                               ("H", nHw, int(colH[w]), nLw // P)):
                    view = tab[0:LO] if sec == "L" else tab[LO:NT]
                    off = 0
                    while off < nsec:
                        ln = min(CH, nsec - off)
                        ix = sb_idx.tile([P, CH // 16], dt.int16, name="ix")
                        nc.sync.dma_start(ix[:, 0:ln // 16],
                                          idxp[:, col0 + off // 16:col0 + (off + ln) // 16])
                        g0 = slot0 + off // P
                        nc.gpsimd.dma_gather(
                            g_t[:, g0:g0 + ln // P, :], view, ix[:, 0:ln // 16],
                            ln, ln, P, single_packet=False,
                            queue_num=conv.qctr % 4)
                        conv.qctr += 1
                        off += ln

                agg = ps_agg.tile([P, P], dt.float32, name="agg")
                ngrp = Sw // P
                for t in range(Tw):
                    jn = min(4, ngrp - t * 4)
                    bs_ps = ps_bs.tile([P, P], dt.float32, name="bs_ps")
                    for jj in range(jn):
                        j = t * 4 + jj
                        nc.tensor.matmul(
                            bs_ps[32 * jj:32 * jj + 32, :], o4_t[:], g_t[:, j, :],
                            start=True, stop=True, tile_position=(0, 32 * jj))
                    Kt = jn * 32
                    bs_sb = sb_bs.tile([P, P], DT, name="bs_sb")
                    nc.vector.tensor_copy(bs_sb[0:Kt, :], bs_ps[0:Kt, :])
                    i2 = sb_i2.tile([P, P], DT, name="i2")
                    r0 = int(i2off[w]) + t * P
                    nc.sync.dma_start(i2[0:Kt, :], ind2p[r0:r0 + Kt, :])
                    nc.tensor.matmul(agg[:], bs_sb[0:Kt, :], i2[0:Kt, :],
                                     start=(t == 0), stop=(t == Tw - 1))

                mT_sb = sb_ms.tile([P, P], DT, name="mT_sb")
                nc.vector.tensor_copy(mT_sb[:], agg[:])
                h_ps = ps_mm.tile([P, P], dt.float32, name="h_ps", tag="mm")
                nc.tensor.matmul(h_ps[:], Wl[:], mT_sb[:], start=True, stop=False)
                nc.tensor.matmul(h_ps[:], Wr[:], in_feat[:, w * P:(w + 1) * P], start=False, stop=True)
                nc.vector.tensor_scalar(out_feat[:, w * P:(w + 1) * P], h_ps[:], bcol[:], 0.0, Alu.add, Alu.max)
                if contrib is not None:
                    rows = min(P, NPC - w * P)
                    hnm_ps = ps_mm.tile([P, P], dt.float32, name="hnm_ps", tag="mm")
                    nc.tensor.transpose(hnm_ps[:], out_feat[:, w * P:(w + 1) * P], id_t[:])
                    hnm_sb = sb_ms.tile([P, P], DT, name="hnm_sb")
                    nc.vector.tensor_copy(hnm_sb[:], hnm_ps[:])
                    nc.scalar.dma_start(contrib[w * P:w * P + rows, :], hnm_sb[0:rows, :])

        def jk(h1, h2, hout, Wlin, bcol, contrib, pool_sb):
            pool_ps = ps_pool.tile([P, G], dt.float32, name="pool_ps")
            for w in range(NW):
                h_ps = ps_mm.tile([P, P], dt.float32, name="jk_ps", tag="mm")
                nc.tensor.matmul(h_ps[:], Wlin[:, 0, :], h1[:, w * P:(w + 1) * P], start=True, stop=False)
                nc.tensor.matmul(h_ps[:], Wlin[:, 1, :], h2[:, w * P:(w + 1) * P], start=False, stop=True)
                nc.vector.tensor_scalar(hout[:, w * P:(w + 1) * P], h_ps[:], bcol[:], 0.0, Alu.add, Alu.max)
                hnm_ps = ps_mm.tile([P, P], dt.float32, name="jknm_ps", tag="mm")
                nc.tensor.transpose(hnm_ps[:], hout[:, w * P:(w + 1) * P], id_t[:])
                hnm_sb = sb_ms.tile([P, P], DT, name="jknm_sb")
                nc.vector.tensor_copy(hnm_sb[:], hnm_ps[:])
                if contrib is not None:
                    rows = min(P, NPC - w * P)
                    nc.scalar.dma_start(contrib[w * P:w * P + rows, :], hnm_sb[0:rows, :])
                pind = sb_i2.tile([P, G], DT, name="pind")
                nc.sync.dma_start(pind[:], poolp[w])
                nc.tensor.matmul(pool_ps[:], hnm_sb[:], pind[:],
                                 start=(w == 0), stop=(w == NW - 1))
            nc.vector.tensor_copy(pool_sb[:], pool_ps[:])

        def allgather(contrib, tab):
            nc.gpsimd.collective_compute(
                "AllGather", Alu.bypass, ins=[contrib[:]], outs=[tab[:]],
                replica_groups=[list(range(NCORES))])

        # ---------------- block 0
        conv(xt, featA, featB, wt["b0_Wl1"], wt["b0_Wr1"], wt["b0_b1"], cA)   # h1
        allgather(cA, tabA)
        if n_convs >= 2:
            conv(tabA, featB, featC, wt["b0_Wl2"], wt["b0_Wr2"], wt["b0_b2"], None)  # h2
            p0_sb = sb.tile([P, G], DT)
            jk(featB, featC, featA, wt["b0_Wlin"], wt["b0_blin"], cB, p0_sb)  # h -> featA
            allgather(cB, tabB)
        if n_convs >= 3:
            conv(tabB, featA, featB, wt["b1_Wl1"], wt["b1_Wr1"], wt["b1_b1"], cA)  # h1'
            allgather(cA, tabC)
        if n_convs >= 4:
            conv(tabC, featB, featC, wt["b1_Wl2"], wt["b1_Wr2"], wt["b1_b2"], None)  # h2'
            p1_sb = sb.tile([P, G], DT)
            jk(featB, featC, featA, wt["b1_Wlin"], wt["b1_blin"], None, p1_sb)

            # ---------------- pooling allreduce + head
            nc.sync.dma_start(pc_in[:, 0:G], p0_sb[:])
            nc.sync.dma_start(pc_in[:, G:2 * G], p1_sb[:])
            nc.gpsimd.collective_compute(
                "AllReduce", Alu.add, ins=[pc_in[:]], outs=[pc_out[:]],
                replica_groups=[list(range(NCORES))])
            pools_sb = sb.tile([P, 2 * G], DT)
            nc.sync.dma_start(pools_sb[:], pc_out[:])

            # BN (folded) per feature chunk
            gbn = sb.tile([P, 2, G], DT)
            for k in range(2):
                nc.vector.tensor_scalar(gbn[:, k, :], pools_sb[:, k * G:(k + 1) * G],
                                        bns_t[:, k:k + 1], bnt_t[:, k:k + 1],
                                        Alu.mult, Alu.add)
            l1_ps = ps_mm.tile([P, G], dt.float32, name="l1_ps", tag="mm")
            for k in range(2):
                nc.tensor.matmul(l1_ps[:], l1w_t[:, k, :], gbn[:, k, :],
                                 start=(k == 0), stop=(k == 1))
            z1 = sb.tile([P, G], DT)
            nc.vector.tensor_scalar(z1[:], l1_ps[:], l1b_t[:], 0.0, Alu.add, Alu.max)
            l2_ps = ps_mm.tile([P, G], dt.float32, name="l2_ps", tag="mm")
            nc.tensor.matmul(l2_ps[0:C, :], l2w_t[:], z1[:], start=True, stop=True)
            z2 = sb.tile([P, G], DT)
            nc.vector.tensor_scalar(z2[0:C, :], l2_ps[0:C, :], l2b_t[0:C, :], None, Alu.add)

            # softmax over C (partition dim) -> transpose to [G, C] first
            for half in range(2):
                zt_ps = ps_mm.tile([P, C], dt.float32, name="zt_ps", tag="mm")
                nc.tensor.transpose(zt_ps[:, 0:C], z2[0:C, half * P:(half + 1) * P], id_t[0:C, 0:C])
                znm = sb.tile([P, C], DT, name=f"znm{half}")
                nc.vector.tensor_copy(znm[:], zt_ps[:, 0:C])
                nmax = sb.tile([P, 1], DT, name=f"nmax{half}")
                nc.vector.tensor_reduce(nmax[:], znm[:], mybir.AxisListType.X, Alu.max, negate=True)
                e_t = sb.tile([P, C], DT, name=f"e_t{half}")
                nc.scalar.activation(e_t[:], znm[:], mybir.ActivationFunctionType.Exp,
                                     bias=nmax[:], scale=1.0)
                ssum = sb.tile([P, 1], DT, name=f"ssum{half}")
                nc.vector.tensor_reduce(ssum[:], e_t[:], mybir.AxisListType.X, Alu.add)
                rcp = sb.tile([P, 1], DT, name=f"rcp{half}")
                nc.vector.reciprocal(rcp[:], ssum[:])
                sm = sb.tile([P, C], DT, name=f"sm{half}")
                nc.vector.tensor_scalar(sm[:], e_t[:], rcp[:], None, Alu.mult)
                nc.sync.dma_start(out[half * P:(half + 1) * P, :], sm[:])

        if debug_tables:
            nc.sync.dma_start(dbgA[:], tabA[:])
            if n_convs >= 2:
                nc.sync.dma_start(dbgB[:], tabB[:])

    nc.compile()
    return nc


# ------------------------------------------------------------------ runtime
def _install_profile_hook():
    try:
        from trn_agent_boot.trn_boot import _ntff_profile_via_ctypes
        hook = _ntff_profile_via_ctypes("/opt/axon/libaxon_pjrt.so")
        m = types.ModuleType("antenv.axon_hooks")
        m.get_axon_ntff_profile_hook = lambda: hook
        sys.modules.setdefault("antenv.axon_hooks", m)
    except Exception:
        pass


def kernel(**inputs):
    from concourse.bass_utils import run_bass_kernel_spmd

    n_convs = int(os.environ.get("KNC_CONVS", "4"))
    debug_tables = bool(int(os.environ.get("KDBG", "0")))
    trace = bool(int(os.environ.get("KTRACE", "0")))
    if trace:
        _install_profile_hook()

    sched, in_maps = _host_inputs(inputs)

    key = (n_convs, debug_tables,
           tuple(int(v) for v in sched["S"][:8]), int(sched["i2rows"]))
    nc = _prog_cache.get(key)
    if nc is None:
        nc = _build_program(sched, n_convs=n_convs, debug_tables=debug_tables)
        _prog_cache[key] = nc

    res = run_bass_kernel_spmd(nc, in_maps, list(range(NCORES)), trace=trace)
    kernel.last_result = res
    out = res.results[0]["out"].astype(np.float32)
    return out



# revision 4
# speedup vs baseline: 1.1687x; 1.1687x over previous
"""GraphSAGE 2-block GNN (nn_BaselineModel_80607946211554) on 8 TRN2 NeuronCores.

Strategy: destination-node sharding, bf16 datapath. Each core owns 6250
contiguous nodes. Node-feature tables are replicated per-core in DRAM in a
slab layout (node n -> row (n//6250)*6272 + n%6250, 22 zero pad rows/slab).
Neighbor mean-aggregation per 128-dst window: dma_gather of bf16 source rows
(edges sorted by dst, host-preprocessed, sections padded to 128 only), then
for each 128-slot tile a DVE-generated indicator (iota==dloc)*inv_deg feeds a
PE matmul accumulating mean^T directly in PSUM. SAGE linears run
feature-major (weights stationary); PSUM->SBUF copies and bias+ReLU run on
the Activation engine. Intermediate tables rebuilt via bf16 AllGather; graph
pooling is a one-hot matmul; the MLP head + softmax is replicated per core.

Self-contained: hardcodes all shapes for the fixed problem instance.
"""
import os
import sys
import types
import numpy as np

N = 50000
E = 1600000
G = 256
F = 128
HID = 128
C = 10
NCORES = 8
NPC = N // NCORES            # 6250 nodes per core
SLAB = 6272                  # slab rows (6250 + 22 zero pad)
NT = NCORES * SLAB           # 50176 table rows
LO = 4 * SLAB                # 25088; table rows < LO hold nodes < 25000
PADROW = 6250                # zero row (local index in both lo/hi views)
P = 128
NW = (NPC + P - 1) // P      # 49 dst windows per core
EPS = 1e-5
GCAP = 168                   # max 128-slot tiles per gather group (5.5MB bf16)

_prog_cache = {}


def _bf16(a):
    import concourse.mybir as mybir
    return np.asarray(a, np.float32).astype(mybir.dt.np(mybir.dt.bfloat16))


def _wrap16(vals):
    """int64 slot values (len mult of 16) -> [128, n/16] int16 wrapped."""
    n = len(vals)
    arr = vals.reshape(n // 16, 16).T.astype(np.int16)   # [16, n/16]
    return np.tile(arr, (8, 1))                           # [128, n/16]


def _wrap128(vals):
    """[S] -> [128, S/128]: slot s -> [s%128, s//128]."""
    return vals.reshape(-1, 128).T.copy()


def _build_schedule(src, dst, invd_full):
    """Static shared schedule + per-core gather index / metadata arrays."""
    core_edges = []
    CL = np.zeros((NCORES, NW), np.int64)
    CH = np.zeros((NCORES, NW), np.int64)
    for c in range(NCORES):
        m = (dst >= c * NPC) & (dst < (c + 1) * NPC)
        s = src[m].astype(np.int64)
        d = (dst[m] - c * NPC).astype(np.int64)
        hi = (s >= N // 2).astype(np.int64)
        w = d >> 7
        order = np.lexsort((d, hi, w))
        s, d, hi, w = s[order], d[order], hi[order], w[order]
        core_edges.append((s, d, hi, w))
        cnt = np.bincount(w * 2 + hi, minlength=NW * 2).reshape(NW, 2)
        CL[c], CH[c] = cnt[:, 0], cnt[:, 1]

    nL = np.maximum(((CL.max(0) + 127) // P) * P, P)
    nH = np.maximum(((CH.max(0) + 127) // P) * P, P)
    gL, gH = nL // P, nH // P                     # tiles per section
    ngrp = gL + gH

    # pack consecutive windows into gather groups of <= GCAP tiles
    groups = []
    cur = []
    cur_cols = 0
    for w in range(NW):
        if cur and cur_cols + ngrp[w] > GCAP:
            groups.append(cur)
            cur, cur_cols = [], 0
        cur.append(w)
        cur_cols += int(ngrp[w])
    if cur:
        groups.append(cur)

    # layouts
    ginfo = []       # per group: dict
    slot_base = np.zeros(NW, np.int64)   # base slot of lo section of window
    hslot_base = np.zeros(NW, np.int64)  # base slot of hi section of window
    col0 = 0         # running tile column over all groups
    for ws in groups:
        colsL = int(gL[ws].sum())
        cols = int(ngrp[ws].sum())
        off = 0
        for w in ws:
            slot_base[w] = (col0 + off) * P
            off += int(gL[w])
        for w in ws:
            hslot_base[w] = (col0 + off) * P
            off += int(gH[w])
        ginfo.append(dict(ws=ws, colsL=colsL, cols=cols, col0=col0))
        col0 += cols
    tot_cols = col0
    S_tot = tot_cols * P

    sched = dict(nL=nL, nH=nH, gL=gL, gH=gH, ngrp=ngrp, groups=ginfo,
                 slot_base=slot_base, hslot_base=hslot_base,
                 tot_cols=tot_cols, S_tot=S_tot,
                 gmax=max(g["cols"] for g in ginfo))

    per_core = []
    for c in range(NCORES):
        s, d, hi, w = core_edges[c]
        # rank within (w, hi) section
        key = w * 2 + hi
        if len(key):
            grp_change = np.r_[True, key[1:] != key[:-1]]
            first_pos = np.flatnonzero(grp_change)
            gidx = np.cumsum(grp_change) - 1
            rank = np.arange(len(d)) - first_pos[gidx]
        else:
            rank = np.zeros(0, np.int64)
        base = np.where(hi == 1, hslot_base[w], slot_base[w])
        pos = base + rank

        trow = (s // NPC) * SLAB + s % NPC
        tval = np.where(hi == 1, trow - LO, trow)

        idx_vals = np.full(S_tot, PADROW, np.int64)
        dloc_vals = np.zeros(S_tot, np.float32)
        val_vals = np.zeros(S_tot, np.float32)
        idx_vals[pos] = tval
        dloc_vals[pos] = (d & 127).astype(np.float32)
        val_vals[pos] = invd_full[c * NPC + d]

        per_core.append(dict(
            idx=_wrap16(idx_vals),
            dloc=_wrap128(dloc_vals),
            val=_wrap128(val_vals),
        ))
    return sched, per_core


def _host_inputs(inputs):
    import concourse.mybir as mybir
    bfnp = mybir.dt.np(mybir.dt.bfloat16)
    f32 = lambda a: np.asarray(a, np.float32)
    x = f32(inputs["x"])
    ei = np.asarray(inputs["edge_index"], np.int64)
    batch = np.asarray(inputs["batch"], np.int64)
    src, dst = ei[0], ei[1]

    deg = np.bincount(dst, minlength=N).astype(np.float32)
    invd_full = (1.0 / np.maximum(deg, 1.0)).astype(np.float32)

    sched, per_core = _build_schedule(src, dst, invd_full)

    xt = np.zeros((NT, F), bfnp)
    xb = _bf16(x)
    for r in range(NCORES):
        xt[r * SLAB:r * SLAB + NPC] = xb[r * NPC:(r + 1) * NPC]

    iota = np.tile(np.arange(P, dtype=np.float32), (P, 1))
    ident = np.eye(P, dtype=np.float32)

    # BN folding
    s_bn = f32(inputs["bn_gamma"]) / np.sqrt(f32(inputs["bn_rv"]) + EPS)
    t_bn = f32(inputs["bn_beta"]) - f32(inputs["bn_rm"]) * s_bn
    bns2 = s_bn.reshape(2, P).T.copy()     # [128, 2]
    bnt2 = t_bn.reshape(2, P).T.copy()

    shared = {
        "xt": xt, "iota": _bf16(iota), "ident": _bf16(ident),
        "bns2": bns2, "bnt2": bnt2,
        "l1w": _bf16(inputs["lin1_W"]), "l1b": f32(inputs["lin1_b"]),
        "l2w": _bf16(inputs["lin2_W"]), "l2b": f32(inputs["lin2_b"]),
    }
    for b in (0, 1):
        for nm in ("Wl1", "Wr1", "Wl2", "Wr2", "Wlin"):
            shared[f"b{b}_{nm}"] = _bf16(inputs[f"b{b}_{nm}"])
        for nm in ("b1", "b2", "blin"):
            shared[f"b{b}_{nm}"] = f32(inputs[f"b{b}_{nm}"])

    in_maps = []
    for c in range(NCORES):
        xoT = np.zeros((F, SLAB), bfnp)
        xoT[:, :NPC] = xb[c * NPC:(c + 1) * NPC].T
        pool_ind = np.zeros((NW, P, G), np.float32)
        bt = batch[c * NPC:(c + 1) * NPC]
        btp = np.full(NW * P, -1, np.int64)
        btp[:NPC] = bt
        btp2 = btp.reshape(NW, P)
        for wi in range(NW):
            vm = btp2[wi] >= 0
            pool_ind[wi, np.arange(P)[vm], btp2[wi][vm]] = 1.0
        im = dict(shared)
        im.update({
            "xoT": xoT, "poolind": _bf16(pool_ind),
            "idx": per_core[c]["idx"],
            "dloc": per_core[c]["dloc"], "val": per_core[c]["val"],
        })
        in_maps.append(im)
    return sched, in_maps


# ------------------------------------------------------------- bass program
def _build_program(sched, n_convs=4, debug_tables=False):
    import concourse.bass as bass
    import concourse.mybir as mybir
    import concourse.tile as tile
    from concourse import bacc
    from concourse import library_config
    from contextlib import ExitStack

    dt = mybir.dt
    DT = dt.float32
    BF = dt.bfloat16
    Alu = mybir.AluOpType
    Act = mybir.ActivationFunctionType

    nL, nH, gL, gH = (sched[k] for k in ("nL", "nH", "gL", "gH"))
    groups = sched["groups"]
    slot_base, hslot_base = sched["slot_base"], sched["hslot_base"]
    GMAX = sched["gmax"]
    TOTC = sched["tot_cols"]

    nc = bacc.Bacc("TRN2", debug=False, num_swdge_queues=4)

    # ---- parameters
    xt = nc.declare_dram_parameter("xt", [NT, F], BF, isOutput=False)
    xoT = nc.declare_dram_parameter("xoT", [F, SLAB], BF, isOutput=False)
    idxp = nc.declare_dram_parameter("idx", [P, TOTC * 8], dt.int16, isOutput=False)
    dlocp = nc.declare_dram_parameter("dloc", [P, TOTC], DT, isOutput=False)
    valp = nc.declare_dram_parameter("val", [P, TOTC], DT, isOutput=False)
    poolp = nc.declare_dram_parameter("poolind", [NW, P, G], BF, isOutput=False)
    iotap = nc.declare_dram_parameter("iota", [P, P], BF, isOutput=False)
    identp = nc.declare_dram_parameter("ident", [P, P], BF, isOutput=False)
    wp = {}
    for b in (0, 1):
        for nm, shp, dty in (("Wl1", [F, HID], BF), ("Wr1", [F, HID], BF),
                             ("b1", [HID], DT),
                             ("Wl2", [HID, HID], BF), ("Wr2", [HID, HID], BF),
                             ("b2", [HID], DT),
                             ("Wlin", [2 * HID, HID], BF), ("blin", [HID], DT)):
            wp[f"b{b}_{nm}"] = nc.declare_dram_parameter(f"b{b}_{nm}", shp, dty, isOutput=False)
    bns2p = nc.declare_dram_parameter("bns2", [P, 2], DT, isOutput=False)
    bnt2p = nc.declare_dram_parameter("bnt2", [P, 2], DT, isOutput=False)
    l1wp = nc.declare_dram_parameter("l1w", [2 * HID, HID], BF, isOutput=False)
    l1bp = nc.declare_dram_parameter("l1b", [HID], DT, isOutput=False)
    l2wp = nc.declare_dram_parameter("l2w", [HID, C], BF, isOutput=False)
    l2bp = nc.declare_dram_parameter("l2b", [C], DT, isOutput=False)

    out = nc.declare_dram_parameter("out", [G, C], DT, isOutput=True)
    if debug_tables:
        dbgA = nc.declare_dram_parameter("dbgA", [NT, F], DT, isOutput=True)
        dbgB = nc.declare_dram_parameter("dbgB", [NT, F], DT, isOutput=True)

    with tile.TileContext(nc) as tc, ExitStack() as ctx:
        sb = ctx.enter_context(tc.tile_pool(name="sb", bufs=1))
        sb_feat = ctx.enter_context(tc.tile_pool(name="sb_feat", bufs=1))
        sb_g = ctx.enter_context(tc.tile_pool(name="sb_g", bufs=2))
        sb_idx = ctx.enter_context(tc.tile_pool(name="sb_idx", bufs=2))
        sb_md = ctx.enter_context(tc.tile_pool(name="sb_md", bufs=4))
        sb_oh = ctx.enter_context(tc.tile_pool(name="sb_oh", bufs=6))
        sb_ms = ctx.enter_context(tc.tile_pool(name="sb_ms", bufs=4))
        sb_pi = ctx.enter_context(tc.tile_pool(name="sb_pi", bufs=3))
        ps_agg = ctx.enter_context(tc.tile_pool(name="ps_agg", bufs=2, space="PSUM"))
        ps_mm = ctx.enter_context(tc.tile_pool(name="ps_mm", bufs=2, space="PSUM"))
        ps_tr = ctx.enter_context(tc.tile_pool(name="ps_tr", bufs=2, space="PSUM"))
        ps_pool = ctx.enter_context(tc.tile_pool(name="ps_pool", bufs=1, space="PSUM"))
        dram = ctx.enter_context(tc.tile_pool(name="dram", bufs=1, space="DRAM"))

        nc.gpsimd.load_library(library_config.mlp)

        # ---- constants into SBUF
        iota_t = sb.tile([P, P], BF)
        nc.sync.dma_start(iota_t[:], iotap[:])
        id_t = sb.tile([P, P], BF)
        nc.sync.dma_start(id_t[:], identp[:])
        wt = {}
        for b in (0, 1):
            for nm in ("Wl1", "Wr1", "Wl2", "Wr2"):
                w_t = sb.tile([P, P], BF, name=f"w{b}{nm}")
                nc.sync.dma_start(w_t[:], wp[f"b{b}_{nm}"][:])
                wt[f"b{b}_{nm}"] = w_t
            wlin_t = sb.tile([P, 2, P], BF, name=f"w{b}lin")
            nc.sync.dma_start(wlin_t[:, 0, :], wp[f"b{b}_Wlin"][0:P, :])
            nc.sync.dma_start(wlin_t[:, 1, :], wp[f"b{b}_Wlin"][P:2 * P, :])
            wt[f"b{b}_Wlin"] = wlin_t
            for nm in ("b1", "b2", "blin"):
                b_t = sb.tile([P, 1], DT, name=f"b{b}{nm}")
                nc.sync.dma_start(b_t[:], wp[f"b{b}_{nm}"][:, None])
                wt[f"b{b}_{nm}"] = b_t
        bns_t = sb.tile([P, 2], DT)
        nc.sync.dma_start(bns_t[:], bns2p[:])
        bnt_t = sb.tile([P, 2], DT)
        nc.sync.dma_start(bnt_t[:], bnt2p[:])
        l1w_t = sb.tile([P, 2, P], BF)
        nc.sync.dma_start(l1w_t[:, 0, :], l1wp[0:P, :])
        nc.sync.dma_start(l1w_t[:, 1, :], l1wp[P:2 * P, :])
        l1b_t = sb.tile([P, 1], DT)
        nc.sync.dma_start(l1b_t[:], l1bp[:, None])
        l2w_t = sb.tile([P, C], BF)
        nc.sync.dma_start(l2w_t[:], l2wp[:])
        l2b_t = sb.tile([P, 1], DT)
        nc.sync.dma_start(l2b_t[0:C, :], l2bp[:, None])

        # feature-major activation buffers [128, SLAB] bf16
        featA = sb_feat.tile([P, SLAB], BF)
        featB = sb_feat.tile([P, SLAB], BF)
        featC = sb_feat.tile([P, SLAB], BF)
        nc.sync.dma_start(featA[:], xoT[:])

        zero_t = sb.tile([P, P], BF)
        nc.vector.memset(zero_t[:], 0.0)

        # DRAM scratch
        cA = dram.tile([SLAB, F], BF)
        cB = dram.tile([SLAB, F], BF)
        tabA = dram.tile([NT, F], BF, addr_space="Shared")
        tabB = dram.tile([NT, F], BF, addr_space="Shared")
        tabC = dram.tile([NT, F], BF, addr_space="Shared")
        pc_in = dram.tile([P, 2 * G], DT)
        pc_out = dram.tile([P, 2 * G], DT, addr_space="Shared")
        nc.sync.dma_start(cA[NPC:SLAB, :], zero_t[0:SLAB - NPC, :])
        nc.sync.dma_start(cB[NPC:SLAB, :], zero_t[0:SLAB - NPC, :])

        def conv(tab, in_feat, out_feat, Wl, Wr, bcol, contrib):
            """One SAGE conv: out_feat[:, n] = relu(mean@Wl + in@Wr + b)."""
            if not hasattr(conv, "qctr"):
                conv.qctr = 0
            for gi in groups:
                ws, colsL, cols, col0 = gi["ws"], gi["colsL"], gi["cols"], gi["col0"]
                g_t = sb_g.tile([P, GMAX, P], BF, name="g_t")
                ix = sb_idx.tile([P, GMAX * 8], dt.int16, name="ix")
                dl = sb_md.tile([P, GMAX], DT, name="dl")
                vl = sb_md.tile([P, GMAX], DT, name="vl")
                nc.sync.dma_start(ix[:, 0:cols * 8],
                                  idxp[:, col0 * 8:(col0 + cols) * 8])
                nc.sync.dma_start(dl[:, 0:cols], dlocp[:, col0:col0 + cols])
                nc.sync.dma_start(vl[:, 0:cols], valp[:, col0:col0 + cols])
                nlo = colsL * P
                nhi = (cols - colsL) * P
                nc.gpsimd.dma_gather(
                    g_t[:, 0:colsL, :], tab[0:LO], ix[:, 0:nlo // 16],
                    nlo, nlo, P, single_packet=False,
                    queue_num=conv.qctr % 4)
                conv.qctr += 1
                nc.gpsimd.dma_gather(
                    g_t[:, colsL:cols, :], tab[LO:NT], ix[:, nlo // 16:cols * 8],
                    nhi, nhi, P, single_packet=False,
                    queue_num=conv.qctr % 4)
                conv.qctr += 1

                for w in ws:
                    agg = ps_agg.tile([P, P], dt.float32, name="agg")
                    # tile columns of this window inside g_t
                    lo0 = (slot_base[w] // P) - col0
                    hi0 = (hslot_base[w] // P) - col0
                    jcols = ([lo0 + k for k in range(int(gL[w]))] +
                             [hi0 + k for k in range(int(gH[w]))])
                    njc = len(jcols)
                    for ji, j in enumerate(jcols):
                        oh = sb_oh.tile([P, P], BF, name="oh")
                        nc.vector.tensor_scalar(
                            oh[:], iota_t[:], dl[:, j:j + 1], vl[:, j:j + 1],
                            Alu.is_equal, Alu.mult)
                        nc.tensor.matmul(agg[:], g_t[:, j, :], oh[:],
                                         start=(ji == 0), stop=(ji == njc - 1))
                    mean_sb = sb_ms.tile([P, P], BF, name="mean_sb")
                    nc.scalar.copy(mean_sb[:], agg[:])
                    h_ps = ps_mm.tile([P, P], dt.float32, name="h_ps", tag="mm")
                    nc.tensor.matmul(h_ps[:], Wl[:], mean_sb[:], start=True, stop=False)
                    nc.tensor.matmul(h_ps[:], Wr[:], in_feat[:, w * P:(w + 1) * P],
                                     start=False, stop=True)
                    nc.scalar.activation(out_feat[:, w * P:(w + 1) * P], h_ps[:],
                                         Act.Relu, bias=bcol[:], scale=1.0)
                    if contrib is not None:
                        rows = min(P, NPC - w * P)
                        hnm_ps = ps_tr.tile([P, P], BF, name="hnm_ps", tag="tr")
                        nc.tensor.transpose(hnm_ps[:], out_feat[:, w * P:(w + 1) * P], id_t[:])
                        hnm_sb = sb_ms.tile([P, P], BF, name="hnm_sb")
                        nc.scalar.copy(hnm_sb[:], hnm_ps[:])
                        nc.scalar.dma_start(contrib[w * P:w * P + rows, :], hnm_sb[0:rows, :])

        def jk(h1, h2, hout, Wlin, bcol, contrib, pool_sb):
            pool_ps = ps_pool.tile([P, G], dt.float32, name="pool_ps")
            for w in range(NW):
                h_ps = ps_mm.tile([P, P], dt.float32, name="jk_ps", tag="mm")
                nc.tensor.matmul(h_ps[:], Wlin[:, 0, :], h1[:, w * P:(w + 1) * P], start=True, stop=False)
                nc.tensor.matmul(h_ps[:], Wlin[:, 1, :], h2[:, w * P:(w + 1) * P], start=False, stop=True)
                nc.scalar.activation(hout[:, w * P:(w + 1) * P], h_ps[:],
                                     Act.Relu, bias=bcol[:], scale=1.0)
                hnm_ps = ps_tr.tile([P, P], BF, name="jknm_ps", tag="tr")
                nc.tensor.transpose(hnm_ps[:], hout[:, w * P:(w + 1) * P], id_t[:])
                hnm_sb = sb_ms.tile([P, P], BF, name="jknm_sb")
                nc.scalar.copy(hnm_sb[:], hnm_ps[:])
                if contrib is not None:
                    rows = min(P, NPC - w * P)
                    nc.scalar.dma_start(contrib[w * P:w * P + rows, :], hnm_sb[0:rows, :])
                pind = sb_pi.tile([P, G], BF, name="pind")
                nc.sync.dma_start(pind[:], poolp[w])
                nc.tensor.matmul(pool_ps[:], hnm_sb[:], pind[:],
                                 start=(w == 0), stop=(w == NW - 1))
            nc.vector.tensor_copy(pool_sb[:], pool_ps[:])

        def allgather(contrib, tab):
            nc.gpsimd.collective_compute(
                "AllGather", Alu.bypass, ins=[contrib[:]], outs=[tab[:]],
                replica_groups=[list(range(NCORES))])

        # ---------------- block 0
        conv(xt, featA, featB, wt["b0_Wl1"], wt["b0_Wr1"], wt["b0_b1"], cA)   # h1
        allgather(cA, tabA)
        if n_convs >= 2:
            conv(tabA, featB, featC, wt["b0_Wl2"], wt["b0_Wr2"], wt["b0_b2"], None)  # h2
            p0_sb = sb.tile([P, G], DT)
            jk(featB, featC, featA, wt["b0_Wlin"], wt["b0_blin"], cB, p0_sb)  # h -> featA
            allgather(cB, tabB)
        if n_convs >= 3:
            conv(tabB, featA, featB, wt["b1_Wl1"], wt["b1_Wr1"], wt["b1_b1"], cA)  # h1'
            allgather(cA, tabC)
        if n_convs >= 4:
            conv(tabC, featB, featC, wt["b1_Wl2"], wt["b1_Wr2"], wt["b1_b2"], None)  # h2'
            p1_sb = sb.tile([P, G], DT)
            jk(featB, featC, featA, wt["b1_Wlin"], wt["b1_blin"], None, p1_sb)

            # ---------------- pooling allreduce + head
            nc.sync.dma_start(pc_in[:, 0:G], p0_sb[:])
            nc.sync.dma_start(pc_in[:, G:2 * G], p1_sb[:])
            nc.gpsimd.collective_compute(
                "AllReduce", Alu.add, ins=[pc_in[:]], outs=[pc_out[:]],
                replica_groups=[list(range(NCORES))])
            pools_sb = sb.tile([P, 2 * G], DT)
            nc.sync.dma_start(pools_sb[:], pc_out[:])

            # BN (folded) per feature chunk -> bf16 for the head matmuls
            gbn = sb.tile([P, 2, G], BF)
            for k in range(2):
                nc.vector.tensor_scalar(gbn[:, k, :], pools_sb[:, k * G:(k + 1) * G],
                                        bns_t[:, k:k + 1], bnt_t[:, k:k + 1],
                                        Alu.mult, Alu.add)
            l1_ps = ps_mm.tile([P, G], dt.float32, name="l1_ps", tag="mm")
            for k in range(2):
                nc.tensor.matmul(l1_ps[:], l1w_t[:, k, :], gbn[:, k, :],
                                 start=(k == 0), stop=(k == 1))
            z1 = sb.tile([P, G], BF)
            nc.vector.tensor_scalar(z1[:], l1_ps[:], l1b_t[:], 0.0, Alu.add, Alu.max)
            l2_ps = ps_mm.tile([P, G], dt.float32, name="l2_ps", tag="mm")
            nc.tensor.matmul(l2_ps[0:C, :], l2w_t[:], z1[:], start=True, stop=True)
            z2 = sb.tile([P, G], DT)
            nc.vector.tensor_scalar(z2[0:C, :], l2_ps[0:C, :], l2b_t[0:C, :], None, Alu.add)

            # softmax over C (partition dim) -> transpose to [G, C] first
            zbf = sb.tile([P, G], BF)
            nc.vector.tensor_copy(zbf[0:C, :], z2[0:C, :])
            for half in range(2):
                zt_ps = ps_mm.tile([P, C], BF, name="zt_ps", tag="mm")
                nc.tensor.transpose(zt_ps[:, 0:C], zbf[0:C, half * P:(half + 1) * P], id_t[0:C, 0:C])
                znm = sb.tile([P, C], DT, name=f"znm{half}")
                nc.vector.tensor_copy(znm[:], zt_ps[:, 0:C])
                nmax = sb.tile([P, 1], DT, name=f"nmax{half}")
                nc.vector.tensor_reduce(nmax[:], znm[:], mybir.AxisListType.X, Alu.max, negate=True)
                e_t = sb.tile([P, C], DT, name=f"e_t{half}")
                nc.scalar.activation(e_t[:], znm[:], Act.Exp,
                                     bias=nmax[:], scale=1.0)
                ssum = sb.tile([P, 1], DT, name=f"ssum{half}")
                nc.vector.tensor_reduce(ssum[:], e_t[:], mybir.AxisListType.X, Alu.add)
                rcp = sb.tile([P, 1], DT, name=f"rcp{half}")
                nc.vector.reciprocal(rcp[:], ssum[:])
                sm = sb.tile([P, C], DT, name=f"sm{half}")
                nc.vector.tensor_scalar(sm[:], e_t[:], rcp[:], None, Alu.mult)
                nc.sync.dma_start(out[half * P:(half + 1) * P, :], sm[:])

        if debug_tables:
            for tabsrc, dbg in ((tabA, dbgA), (tabB, dbgB)):
                for r in range(NT // P):
                    st = sb_ms.tile([P, F], BF, name="dbg_st")
                    nc.sync.dma_start(st[:], tabsrc[r * P:(r + 1) * P, :])
                    st2 = sb_ms.tile([P, F], DT, name="dbg_st2")
                    nc.vector.tensor_copy(st2[:], st[:])
                    nc.sync.dma_start(dbg[r * P:(r + 1) * P, :], st2[:])

    nc.compile()
    return nc


# ------------------------------------------------------------------ runtime
def _install_profile_hook():
    try:
        from trn_agent_boot.trn_boot import _ntff_profile_via_ctypes
        hook = _ntff_profile_via_ctypes("/opt/axon/libaxon_pjrt.so")
        m = types.ModuleType("antenv.axon_hooks")
        m.get_axon_ntff_profile_hook = lambda: hook
        sys.modules.setdefault("antenv.axon_hooks", m)
    except Exception:
        pass


def kernel(**inputs):
    from concourse.bass_utils import run_bass_kernel_spmd

    n_convs = int(os.environ.get("KNC_CONVS", "4"))
    debug_tables = bool(int(os.environ.get("KDBG", "0")))
    trace = bool(int(os.environ.get("KTRACE", "0")))
    if trace:
        _install_profile_hook()

    sched, in_maps = _host_inputs(inputs)

    key = (n_convs, debug_tables, int(sched["tot_cols"]), int(sched["gmax"]))
    nc = _prog_cache.get(key)
    if nc is None:
        nc = _build_program(sched, n_convs=n_convs, debug_tables=debug_tables)
        _prog_cache[key] = nc

    res = run_bass_kernel_spmd(nc, in_maps, list(range(NCORES)), trace=trace)
    kernel.last_result = res
    out = res.results[0]["out"].astype(np.float32)
    return out
